# revision 1
# baseline (speedup 1.0000x reference)
"""Trainium2 Bass kernel for ConcatHandshaking.

out[b, p, :] = tanh(hidden[b, i_p] @ W1.T + hidden[b, j_p] @ W2.T + fc_b)
for the S*(S+1)/2 upper-triangular pairs (i_p, j_p), i-major order.

Device layout: output features (H=768) on SBUF partitions, pair index on the
free dim.  Then the pair-add is `q2T[:, j] + p1T[:, i]` where the second term
is a per-partition scalar -> one DVE tensor_scalar_add per triu segment,
fused bias, one big ACT tanh per output chunk, large contiguous DMA writes.

Sharding (8 cores): core k handles batch b = k//2 and output-feature rows
[384*(k%2), 384*(k%2)+384) -> 3 stripes of [128 features, 32896 pairs] each.
Per-core DRAM output is (3, 128, 32896); host reassembles + transposes.

Matmul operands ship as one bf16 tensor (PE 4x faster than f32; rel err
~1e-3 after f32 PSUM accumulation); fcb/zeros ship in a tiny f32 tensor.
The first stripe uses small leading chunks so the first output DMA starts
~12us in instead of waiting on a full 8224-wide chunk.
"""

import sys

import numpy as np

for _p in ("/opt/trn_rl_repo",):
    if _p not in sys.path:
        sys.path.insert(0, _p)

B, S, H = 4, 256, 768
P = S * (S + 1) // 2  # 32896
KT = H // 128  # 6 k-tiles
OC = 3  # o-chunks (of 128) per core
# bf16 packed matmul input columns: [ ht (S) | w1t (384) | w2t (384) ]
W1C = S
W2C = S + 128 * OC
IC16 = S + 2 * 128 * OC  # 1024
BIGCHUNK = 2056
SMALL = 2056
# segments with i < FUSE_T run as single ACT ops (tanh with per-partition
# bias = p1[:, i]) writing ot2 directly -- no DVE pass, no extra SBUF hops.
# Short segments (i >= FUSE_T) would drown in ACT instruction overhead, so
# they keep the add + one-big-tanh path on DVE.  (A GPSIMD band was tried
# and is ~6x slower per op on real HW than the cost model claims -- unused.)
# Consecutive full segments are merged in PAIRS into one DVE tensor_tensor
# with an overlapping-window AP: row g reads q2[i+g : i+g+L], adds
# p1[:, i+g] (free-step-0 broadcast).  Row 1 writes one spill element that
# the next instruction's first write repairs (same-engine program order).
FUSE_T = 32
GPS_LO = 224
GPS_HI = 224

_NC_CACHE = {}
LAST = {}


def _stripe_chunks(c):
    if c == 0:
        # small leading chunks: first output DMA launches early and the
        # stream never stalls waiting on one big chunk's DVE+ACT latency
        return [1028, 1028] + [BIGCHUNK] * 15
    return [BIGCHUNK] * 16


def _chunk_pieces(chunk_list):
    """Split triu segments along chunk boundaries.

    Returns per-chunk lists of (i, src0, src1, dst0):
    chunk[:, dst0:dst0+(src1-src0)] = q2T[:, src0:src1] + p1T[:, i].
    """
    bounds = [0]
    for sz in chunk_list:
        bounds.append(bounds[-1] + sz)
    assert bounds[-1] == P
    pieces = [[] for _ in chunk_list]
    off = 0
    for i in range(S):
        seg0, seg1 = off, off + (S - i)
        off = seg1
        for ci, (c0, c1) in enumerate(zip(bounds[:-1], bounds[1:])):
            s = max(seg0, c0)
            e = min(seg1, c1)
            if e > s:
                src0 = i + (s - seg0)  # free index in q2T is j itself
                pieces[ci].append((i, src0, src0 + (e - s), s - c0))
    return pieces


def _build_nc(loop_k=None, fuse_t=None, gps_lo=None, gps_hi=None):
    if fuse_t is None:
        fuse_t = FUSE_T
    if gps_lo is None:
        gps_lo = GPS_LO
    if gps_hi is None:
        gps_hi = GPS_HI
    import contextlib

    import concourse.bacc as bacc
    import concourse.bass as bass
    import concourse.mybir as mybir
    import concourse.tile as tile

    def _sub_ap(t, off, dims):
        return bass.AP(tensor=t.tensor, offset=t.offset + off, ap=[t.ap[0]] + dims)

    f32 = mybir.dt.float32
    bf16 = mybir.dt.bfloat16
    # Bacc (not raw Bass): its compile() runs generate_event_semaphores,
    # which splits multi-sem waits to satisfy TRN2's 1-wait-per-instruction.
    nc = bacc.Bacc()

    inp16_d = nc.declare_dram_parameter("inp16", [H, IC16], bf16, isOutput=False)
    # f32 side data: col 0 = fcb (rows 0:384), col 1 = zeros
    aux_d = nc.declare_dram_parameter("aux", [H, 2], f32, isOutput=False)
    out_d = nc.declare_dram_parameter("out", [OC, 128, P], f32, isOutput=True)

    Tanh = mybir.ActivationFunctionType.Tanh

    with tile.TileContext(nc) as tc:
        with (
            tc.tile_pool(name="const", bufs=1) as cpool,
            tc.tile_pool(name="mm", bufs=2, space="PSUM") as mpool,
            tc.tile_pool(name="outp", bufs=6) as opool,
            tc.tile_pool(name="outp2", bufs=12) as opool2,
            tc.For_i(0, loop_k, 1) if loop_k else contextlib.nullcontext(),
        ):
            # one DMA per k-tile so matmul kk can start as soon as its
            # k-tile lands (pipelines the load under the matmul chain)
            inp_b = cpool.tile([128, KT * IC16], bf16, name="inp_b")
            for kk in range(KT):
                nc.sync.dma_start(
                    inp_b[:, kk * IC16 : (kk + 1) * IC16],
                    inp16_d[kk * 128 : (kk + 1) * 128, :],
                )
            aux_b = cpool.tile([128, KT * 2], f32, name="aux_b")
            nc.sync.dma_start(
                aux_b[:].rearrange("p (t c) -> p t c", t=KT),
                aux_d.rearrange("(t p) c -> p t c", p=128),
            )
            # block kk occupies cols [kk*IC16, (kk+1)*IC16)
            ht_t = [inp_b[:, kk * IC16 : kk * IC16 + S] for kk in range(KT)]
            fcb_t = [aux_b[:, c * 2 : c * 2 + 1] for c in range(OC)]

            for c in range(OC):
                pm1 = mpool.tile([128, S], f32, name="pm1")
                pm2 = mpool.tile([128, S], f32, name="pm2")
                for kk in range(KT):
                    nc.tensor.matmul(
                        pm1[:],
                        inp_b[
                            :, kk * IC16 + W1C + c * 128 : kk * IC16 + W1C + (c + 1) * 128
                        ],
                        ht_t[kk],
                        start=(kk == 0),
                        stop=(kk == KT - 1),
                    )
                for kk in range(KT):
                    nc.tensor.matmul(
                        pm2[:],
                        inp_b[
                            :, kk * IC16 + W2C + c * 128 : kk * IC16 + W2C + (c + 1) * 128
                        ],
                        ht_t[kk],
                        start=(kk == 0),
                        stop=(kk == KT - 1),
                    )
                p1 = cpool.tile([128, S], f32, name=f"p1_{c}")
                # one pad column: paired adds read q2[:, i+L] (one past the
                # segment) whose result only lands in the repaired spill cell
                q2 = cpool.tile([128, S + 1], f32, name=f"q2_{c}")
                nc.vector.tensor_copy(p1[:], pm1[:])
                nc.vector.tensor_scalar_add(q2[:, :S], pm2[:], fcb_t[c])

                chunk_list = _stripe_chunks(c)
                pieces = _chunk_pieces(chunk_list)
                coff = 0
                for ci, csz in enumerate(chunk_list):
                    fused = [
                        p for p in pieces[ci]
                        if p[0] < fuse_t or p[0] >= gps_lo
                    ]
                    rest = [
                        p for p in pieces[ci]
                        if fuse_t <= p[0] < gps_lo
                    ]
                    ot2 = opool2.tile([128, BIGCHUNK], f32, name="ot2")
                    if rest:
                        # adds for the short segments (paired where legal),
                        # then one tanh over their contiguous extent
                        ot = opool.tile([128, BIGCHUNK], f32, name="ot")
                        k = 0
                        while k < len(rest):
                            i, s0, s1, d0 = rest[k]
                            L = s1 - s0
                            pair = False
                            if False and k + 1 < len(rest) and s0 == i and L == S - i:
                                i2, t0, t1, e0 = rest[k + 1]
                                pair = (
                                    i2 == i + 1
                                    and t0 == i2
                                    and (t1 - t0) == (S - i2)
                                    and e0 == d0 + L
                                    and (k + 2 < len(rest) or csz < BIGCHUNK)
                                )
                            if pair:
                                nc.vector.tensor_tensor(
                                    _sub_ap(ot, d0, [[L, 2], [1, L]]),
                                    _sub_ap(q2, s0, [[1, 2], [1, L]]),
                                    _sub_ap(p1, i, [[1, 2], [0, L]]),
                                    op=mybir.AluOpType.add,
                                )
                                k += 2
                            else:
                                nc.vector.tensor_scalar_add(
                                    ot[:, d0 : d0 + L],
                                    q2[:, s0:s1],
                                    p1[:, i : i + 1],
                                )
                                k += 1
                        r0 = rest[0][3]
                        r1 = rest[-1][3] + (rest[-1][2] - rest[-1][1])
                        nc.scalar.activation(ot2[:, r0:r1], ot[:, r0:r1], Tanh)
                    for (i, s0, s1, d0) in fused:
                        nc.scalar.activation(
                            ot2[:, d0 : d0 + (s1 - s0)],
                            q2[:, s0:s1],
                            Tanh,
                            bias=p1[:, i : i + 1],
                        )
                    nc.sync.dma_start(
                        out_d[c, :, coff : coff + csz], ot2[:, :csz]
                    )
                    coff += csz
    nc.compile()
    return nc


def _get_nc():
    if "nc" not in _NC_CACHE:
        _NC_CACHE["nc"] = _build_nc()
    return _NC_CACHE["nc"]


def _make_in_maps(hidden_state, fc_w, fc_b):
    import ml_dtypes

    in_maps = []
    for k in range(8):
        b, h0 = k // 2, 384 * (k % 2)
        inp16 = np.empty((H, IC16), dtype=ml_dtypes.bfloat16)
        inp16[:, :S] = hidden_state[b].T.astype(ml_dtypes.bfloat16)
        inp16[:, W1C : W1C + 384] = fc_w[h0 : h0 + 384, :H].T.astype(
            ml_dtypes.bfloat16
        )
        inp16[:, W2C : W2C + 384] = fc_w[h0 : h0 + 384, H:].T.astype(
            ml_dtypes.bfloat16
        )
        aux = np.zeros((H, 2), dtype=np.float32)
        aux[: 128 * OC, 0] = fc_b[h0 : h0 + 384]
        in_maps.append(dict(inp16=inp16, aux=aux))
    return in_maps


def kernel(hidden_state, fc_w, fc_b, _trace=False, **_trace_kwargs):
    from concourse.bass_utils import run_bass_kernel_spmd

    hidden_state = np.asarray(hidden_state, dtype=np.float32)
    fc_w = np.asarray(fc_w, dtype=np.float32)
    fc_b = np.asarray(fc_b, dtype=np.float32)

    in_maps = _make_in_maps(hidden_state, fc_w, fc_b)
    nc = _get_nc()
    res = run_bass_kernel_spmd(
        nc, in_maps, core_ids=list(range(8)), trace=_trace, **_trace_kwargs
    )
    LAST["res"] = res

    full = np.empty((B, H, P), dtype=np.float32)
    for k in range(8):
        b, h0 = k // 2, 384 * (k % 2)
        full[b, h0 : h0 + 384] = res.results[k]["out"].reshape(384, P)
    return np.ascontiguousarray(full.transpose(0, 2, 1))



# revision 2
# speedup vs baseline: 1.2013x; 1.2013x over previous
"""Trainium2 Bass kernel for ConcatHandshaking.

out[b, p, :] = tanh(hidden[b, i_p] @ W1.T + hidden[b, j_p] @ W2.T + fc_b)
for the S*(S+1)/2 upper-triangular pairs (i_p, j_p), i-major order.

Device layout: output features (H=768) on SBUF partitions, pair index on the
free dim.  The pair dimension is emitted DIAGONAL-major: for diagonal
d = j - i, out(:, i, i+d) = p1T[:, i] + q2T[:, i+d] is an elementwise add of
two contiguous windows -- no broadcast operand.  G=8 consecutive diagonals
are blocked into ONE DVE tensor_tensor via a 3D access pattern (row g reads
p1[0:L] and q2[d0+g : d0+g+L]); rows keep the max length L = 256-d0, so rows
g>0 write g pad columns at their tail.  Everything (p1, q2, adds, tanh,
output DMA) runs in bf16: the adds qualify for the DVE 2x/4x packed modes
and the output DMA halves vs f32.  The host drops pad columns and restores
triu order with one precomputed gather, then converts to f32.

Sharding (8 cores): core k handles batch b = k//2 and output-feature rows
[384*(k%2), 384*(k%2)+384) -> 3 stripes of [128 features, PPAD cols] each.

Matmul operands ship as one bf16 tensor (PE 4x faster than f32; rel err
~3e-3 after f32 PSUM accumulation); fcb/zeros ship in a tiny f32 tensor.
"""

import sys

import numpy as np

for _p in ("/opt/trn_rl_repo",):
    if _p not in sys.path:
        sys.path.insert(0, _p)

B, S, H = 4, 256, 768
P = S * (S + 1) // 2  # 32896
KT = H // 128  # 6 k-tiles
OC = 3  # o-chunks (of 128) per core
# bf16 packed matmul input columns: [ ht (S) | w1t (384) | w2t (384) ]
W1C = S
W2C = S + 128 * OC
IC16 = S + 2 * 128 * OC  # 1024

G = 8  # diagonals per DVE tensor_tensor block
NBLK = S // G  # 32 blocks per stripe
# block b covers diagonals [G*b, G*b+G), padded row length L_b = S - G*b
BLK_L = [S - G * b for b in range(NBLK)]
BLK_COLS = [G * L for L in BLK_L]
BLK_BASE = np.concatenate([[0], np.cumsum(BLK_COLS)]).astype(np.int64)
PPAD = int(BLK_BASE[-1])  # 33792

# group whole blocks into chunks of ~TARGET cols for ACT + output DMA
TARGET = 4000


def _chunks():
    """List of (block_lo, block_hi, col_off, n_cols); first chunk is a single
    block so the first output DMA launches early."""
    chunks = []
    b = 0
    first = True
    while b < NBLK:
        e = b + 1
        if not first:
            while e < NBLK and BLK_BASE[e] - BLK_BASE[b] < TARGET:
                e += 1
        first = False
        chunks.append((b, e, int(BLK_BASE[b]), int(BLK_BASE[e] - BLK_BASE[b])))
        b = e
    return chunks

CHUNKS = _chunks()
CMAX = max(c[3] for c in CHUNKS)

_NC_CACHE = {}
LAST = {}


def _build_nc():
    import concourse.bacc as bacc
    import concourse.bass as bass
    import concourse.mybir as mybir
    import concourse.tile as tile

    def _sub_ap(t, off, dims):
        return bass.AP(tensor=t.tensor, offset=t.offset + off, ap=[t.ap[0]] + dims)

    f32 = mybir.dt.float32
    bf16 = mybir.dt.bfloat16
    # Bacc (not raw Bass): its compile() runs generate_event_semaphores,
    # which splits multi-sem waits to satisfy TRN2's 1-wait-per-instruction.
    nc = bacc.Bacc()

    inp16_d = nc.declare_dram_parameter("inp16", [H, IC16], bf16, isOutput=False)
    # f32 side data: col 0 = fcb (rows 0:384), col 1 = zeros
    aux_d = nc.declare_dram_parameter("aux", [H, 2], f32, isOutput=False)
    out_d = nc.declare_dram_parameter("out", [OC, 128, PPAD], bf16, isOutput=True)

    Tanh = mybir.ActivationFunctionType.Tanh

    with tile.TileContext(nc) as tc:
        with (
            tc.tile_pool(name="const", bufs=1) as cpool,
            tc.tile_pool(name="mm", bufs=4, space="PSUM") as mpool,
            tc.tile_pool(name="outp", bufs=4) as opool,
            tc.tile_pool(name="outp2", bufs=4) as opool2,
        ):
            # one DMA per k-tile so matmul kk can start as soon as its
            # k-tile lands (pipelines the load under the matmul chain)
            inp_b = cpool.tile([128, KT * IC16], bf16, name="inp_b")
            for kk in range(KT):
                nc.sync.dma_start(
                    inp_b[:, kk * IC16 : (kk + 1) * IC16],
                    inp16_d[kk * 128 : (kk + 1) * 128, :],
                )
            aux_b = cpool.tile([128, KT * 2], f32, name="aux_b")
            nc.sync.dma_start(
                aux_b[:].rearrange("p (t c) -> p t c", t=KT),
                aux_d.rearrange("(t p) c -> p t c", p=128),
            )
            # block kk occupies cols [kk*IC16, (kk+1)*IC16)
            ht_t = [inp_b[:, kk * IC16 : kk * IC16 + S] for kk in range(KT)]
            fcb_t = [aux_b[:, c * 2 : c * 2 + 1] for c in range(OC)]

            for c in range(OC):
                pm1 = mpool.tile([128, S], f32, name="pm1")
                pm2 = mpool.tile([128, S], f32, name="pm2")
                for kk in range(KT):
                    nc.tensor.matmul(
                        pm1[:],
                        inp_b[
                            :, kk * IC16 + W1C + c * 128 : kk * IC16 + W1C + (c + 1) * 128
                        ],
                        ht_t[kk],
                        start=(kk == 0),
                        stop=(kk == KT - 1),
                    )
                for kk in range(KT):
                    nc.tensor.matmul(
                        pm2[:],
                        inp_b[
                            :, kk * IC16 + W2C + c * 128 : kk * IC16 + W2C + (c + 1) * 128
                        ],
                        ht_t[kk],
                        start=(kk == 0),
                        stop=(kk == KT - 1),
                    )
                p1 = cpool.tile([128, S], bf16, name=f"p1_{c}")
                # G pad columns: row g of a block reads q2 up to col 255+g
                q2 = cpool.tile([128, S + G], bf16, name=f"q2_{c}")
                nc.vector.memset(q2[:, S : S + G], 0.0)
                nc.vector.tensor_copy(p1[:], pm1[:])
                nc.vector.tensor_scalar_add(q2[:, :S], pm2[:], fcb_t[c])

                for (blo, bhi, coff, csz) in CHUNKS:
                    ot = opool.tile([128, CMAX], bf16, name="ot")
                    for bb in range(blo, bhi):
                        L = BLK_L[bb]
                        d0 = G * bb
                        off = int(BLK_BASE[bb]) - coff
                        nc.vector.tensor_tensor(
                            _sub_ap(ot, off, [[L, G], [1, L]]),
                            _sub_ap(p1, 0, [[0, G], [1, L]]),
                            _sub_ap(q2, d0, [[1, G], [1, L]]),
                            op=mybir.AluOpType.add,
                        )
                    ot2 = opool2.tile([128, CMAX], bf16, name="ot2")
                    nc.scalar.activation(ot2[:, :csz], ot[:, :csz], Tanh)
                    nc.sync.dma_start(out_d[c, :, coff : coff + csz], ot2[:, :csz])
    nc.compile()
    return nc


def _get_nc():
    if "nc" not in _NC_CACHE:
        _NC_CACHE["nc"] = _build_nc()
    return _NC_CACHE["nc"]


def _make_in_maps(hidden_state, fc_w, fc_b):
    import ml_dtypes

    in_maps = []
    for k in range(8):
        b, h0 = k // 2, 384 * (k % 2)
        inp16 = np.empty((H, IC16), dtype=ml_dtypes.bfloat16)
        inp16[:, :S] = hidden_state[b].T.astype(ml_dtypes.bfloat16)
        inp16[:, W1C : W1C + 384] = fc_w[h0 : h0 + 384, :H].T.astype(
            ml_dtypes.bfloat16
        )
        inp16[:, W2C : W2C + 384] = fc_w[h0 : h0 + 384, H:].T.astype(
            ml_dtypes.bfloat16
        )
        aux = np.zeros((H, 2), dtype=np.float32)
        aux[: 128 * OC, 0] = fc_b[h0 : h0 + 384]
        in_maps.append(dict(inp16=inp16, aux=aux))
    return in_maps


def _devcol():
    """Map triu pair index p -> device (diagonal-major padded) column."""
    ii, jj = np.triu_indices(S)
    d = jj - ii
    blk = d // G
    g = d % G
    L = S - G * blk
    return BLK_BASE[blk] + g * L + ii


_DEVCOL = _devcol()


def kernel(hidden_state, fc_w, fc_b, _trace=False, **_trace_kwargs):
    from concourse.bass_utils import run_bass_kernel_spmd

    hidden_state = np.asarray(hidden_state, dtype=np.float32)
    fc_w = np.asarray(fc_w, dtype=np.float32)
    fc_b = np.asarray(fc_b, dtype=np.float32)

    in_maps = _make_in_maps(hidden_state, fc_w, fc_b)
    nc = _get_nc()
    res = run_bass_kernel_spmd(
        nc, in_maps, core_ids=list(range(8)), trace=_trace, **_trace_kwargs
    )
    LAST["res"] = res

    full = np.empty((B, H, P), dtype=np.float32)
    for k in range(8):
        b, h0 = k // 2, 384 * (k % 2)
        dev = res.results[k]["out"].reshape(384, PPAD)
        full[b, h0 : h0 + 384] = dev[:, _DEVCOL].astype(np.float32)
    return np.ascontiguousarray(full.transpose(0, 2, 1))


# revision 5
# speedup vs baseline: 1.4523x; 1.2089x over previous
"""Trainium2 Bass kernel for ConcatHandshaking.

out[b, p, :] = tanh(hidden[b, i_p] @ W1.T + hidden[b, j_p] @ W2.T + fc_b)
for the S*(S+1)/2 upper-triangular pairs (i_p, j_p), i-major order.

Device layout: output features (H=768) on SBUF partitions, pair index on the
free dim.  The pair dimension is emitted DIAGONAL-major: for diagonal
d = j - i, out(:, i, i+d) = p1T[:, i] + q2T[:, i+d] is an elementwise add of
two contiguous windows -- no broadcast operand.  G consecutive diagonals are
blocked into ONE DVE tensor_tensor via a 3D access pattern (row g reads
p1[0:L] and q2[d0+g : d0+g+L]); rows keep the max length L = 256-d0, so row
g writes g pad columns at its tail.  Everything (p1, q2, adds, tanh, output
DMA) runs in bf16: the adds qualify for the DVE 2x packed mode and the
output DMA halves vs f32.  The host drops pad columns and restores triu
order with one precomputed gather, then converts to f32.

Lead-in optimizations: the input tensor is column-grouped [ht | w_c0 | w_c1
| w_c2] so three small DMAs deliver exactly what stripe 0's matmuls need
first; stripe c+1's matmuls carry a nosync dep on stripe c's PSUM stops so
the scheduler cannot interleave all stripes k-tile-major (which would delay
stripe 0's PSUM -> first tanh by ~5us); the first two blocks are G=4 and
chunked alone so the first tanh + output DMA launch early.

Sharding (8 cores): core k handles batch b = k//2 and output-feature rows
[384*(k%2), 384*(k%2)+384) -> 3 stripes of [128 features, PPAD cols] each.
"""

import sys

import numpy as np

for _p in ("/opt/trn_rl_repo",):
    if _p not in sys.path:
        sys.path.insert(0, _p)

B, S, H = 4, 256, 768
P = S * (S + 1) // 2  # 32896
KT = H // 128  # 6 k-tiles
OC = 3  # o-chunks (of 128) per core
# bf16 packed matmul input columns: [ ht (S) | w1_c0 w2_c0 | w1_c1 w2_c1 | ... ]
IC16 = S + 2 * 128 * OC  # 1024

GPAD = 8  # q2 pad columns (max G)
# diagonal blocks (d0, G, L): two G=4 leaders for a fast first tanh, then G=8
BLOCKS = [(0, 4, 256), (4, 4, 252)] + [
    (8 + 8 * t, 8, 248 - 8 * t) for t in range(31)
]
_bases = np.concatenate([[0], np.cumsum([g * l for (_, g, l) in BLOCKS])])
BLK_BASE = _bases.astype(np.int64)
PPAD = int(BLK_BASE[-1])  # 33776

TARGET = 3500  # chunk col target for ACT + output DMA granularity


def _chunks():
    """(block_lo, block_hi, col_off, n_cols) groups; first two chunks are the
    single G=4 blocks (early first DMA), last chunk split small (short drain)."""
    chunks = [(0, 1, 0, int(BLK_BASE[1] - BLK_BASE[0])),
              (1, 2, int(BLK_BASE[1]), int(BLK_BASE[2] - BLK_BASE[1]))]
    b = 2
    while b < len(BLOCKS):
        e = b + 1
        while e < len(BLOCKS) and BLK_BASE[e] - BLK_BASE[b] < TARGET:
            e += 1
        chunks.append((b, e, int(BLK_BASE[b]), int(BLK_BASE[e] - BLK_BASE[b])))
        b = e
    # split the trailing chunk while it is large, so the final output DMA
    # (the pipeline drain) moves less than ~2000 cols
    while chunks[-1][3] > 2000:
        blo, bhi, coff, csz = chunks.pop()
        mid = blo + 1
        while BLK_BASE[mid] - BLK_BASE[blo] < csz // 2:
            mid += 1
        chunks.append((blo, mid, coff, int(BLK_BASE[mid] - BLK_BASE[blo])))
        chunks.append((mid, bhi, int(BLK_BASE[mid]), int(BLK_BASE[bhi] - BLK_BASE[mid])))
    return chunks


CHUNKS = _chunks()
CMAX = max(c[3] for c in CHUNKS)

_NC_CACHE = {}
LAST = {}


def _build_nc():
    import bass_rust
    import concourse.bacc as bacc
    import concourse.bass as bass
    import concourse.mybir as mybir
    import concourse.tile as tile

    def _sub_ap(t, off, dims):
        return bass.AP(tensor=t.tensor, offset=t.offset + off, ap=[t.ap[0]] + dims)

    f32 = mybir.dt.float32
    bf16 = mybir.dt.bfloat16
    nc = bacc.Bacc()

    inp16_d = nc.declare_dram_parameter("inp16", [H, IC16], bf16, isOutput=False)
    # f32 side data: col 0 = fcb (rows 0:384), col 1 = zeros
    aux_d = nc.declare_dram_parameter("aux", [H, 2], f32, isOutput=False)
    out_d = nc.declare_dram_parameter("out", [OC, 128, PPAD], bf16, isOutput=True)

    Tanh = mybir.ActivationFunctionType.Tanh

    with tile.TileContext(nc) as tc:
        with (
            tc.tile_pool(name="const", bufs=1) as cpool,
            tc.tile_pool(name="mm", bufs=4, space="PSUM") as mpool,
            tc.tile_pool(name="outp", bufs=4) as opool,
            tc.tile_pool(name="outp2", bufs=4) as opool2,
        ):
            inp_b = cpool.tile([128, KT * IC16], bf16, name="inp_b")
            inp_r = inp_b[:].rearrange("p (t c) -> p t c", t=KT)
            src_r = inp16_d.rearrange("(t p) c -> p t c", p=128)
            # part A: ht + stripe-0 weights, two k-tiles per DMA so PE can
            # start as soon as the first pair lands
            for kk in range(0, KT, 2):
                nc.sync.dma_start(
                    inp_r[:, kk : kk + 2, 0:512], src_r[:, kk : kk + 2, 0:512]
                )
            aux_b = cpool.tile([128, KT * 2], f32, name="aux_b")
            nc.sync.dma_start(
                aux_b[:].rearrange("p (t c) -> p t c", t=KT),
                aux_d.rearrange("(t p) c -> p t c", p=128),
            )
            # part B: stripe 1-2 weights, one DMA
            nc.sync.dma_start(
                inp_r[:, :, 512:IC16], src_r[:, :, 512:IC16]
            )

            ht_t = [inp_b[:, kk * IC16 : kk * IC16 + S] for kk in range(KT)]
            fcb_t = [aux_b[:, c * 2 : c * 2 + 1] for c in range(OC)]

            prev_stops = []
            for c in range(OC):
                w1c = S + 256 * c
                w2c = S + 256 * c + 128
                pm1 = mpool.tile([128, S], f32, name="pm1")
                pm2 = mpool.tile([128, S], f32, name="pm2")
                stops = []
                for pm, wc in ((pm1, w1c), (pm2, w2c)):
                    for kk in range(KT):
                        mm = nc.tensor.matmul(
                            pm[:],
                            inp_b[:, kk * IC16 + wc : kk * IC16 + wc + 128],
                            ht_t[kk],
                            start=(kk == 0),
                            stop=(kk == KT - 1),
                        )
                        if kk == 0 and prev_stops:
                            # keep PE stripe-major: without this the scheduler
                            # interleaves all stripes k-tile-major and stripe
                            # 0's PSUM stop retires ~5us late
                            deps = bass_rust.InstructionNameOrderedSet()
                            for nm in prev_stops:
                                deps.add(nm)
                            mm.ins.add_nosync_dependencies_from(deps)
                        if kk == KT - 1:
                            stops.append(mm.ins.name)
                prev_stops = stops

                p1 = cpool.tile([128, S], bf16, name=f"p1_{c}")
                q2 = cpool.tile([128, S + GPAD], bf16, name=f"q2_{c}")
                nc.vector.memset(q2[:, S : S + GPAD], 0.0)
                nc.vector.tensor_copy(p1[:], pm1[:])
                nc.vector.tensor_scalar_add(q2[:, :S], pm2[:], fcb_t[c])

                for (blo, bhi, coff, csz) in CHUNKS:
                    ot = opool.tile([128, CMAX], bf16, name="ot")
                    for bb in range(blo, bhi):
                        d0, G, L = BLOCKS[bb]
                        off = int(BLK_BASE[bb]) - coff
                        nc.vector.tensor_tensor(
                            _sub_ap(ot, off, [[L, G], [1, L]]),
                            _sub_ap(p1, 0, [[0, G], [1, L]]),
                            _sub_ap(q2, d0, [[1, G], [1, L]]),
                            op=mybir.AluOpType.add,
                        )
                    ot2 = opool2.tile([128, CMAX], bf16, name="ot2")
                    nc.scalar.activation(ot2[:, :csz], ot[:, :csz], Tanh)
                    nc.sync.dma_start(out_d[c, :, coff : coff + csz], ot2[:, :csz])
    nc.compile()
    return nc


def _get_nc():
    if "nc" not in _NC_CACHE:
        _NC_CACHE["nc"] = _build_nc()
    return _NC_CACHE["nc"]


def _make_in_maps(hidden_state, fc_w, fc_b):
    import ml_dtypes

    in_maps = []
    for k in range(8):
        b, h0 = k // 2, 384 * (k % 2)
        inp16 = np.empty((H, IC16), dtype=ml_dtypes.bfloat16)
        inp16[:, :S] = hidden_state[b].T.astype(ml_dtypes.bfloat16)
        for c in range(OC):
            r0 = h0 + 128 * c
            inp16[:, S + 256 * c : S + 256 * c + 128] = fc_w[
                r0 : r0 + 128, :H
            ].T.astype(ml_dtypes.bfloat16)
            inp16[:, S + 256 * c + 128 : S + 256 * c + 256] = fc_w[
                r0 : r0 + 128, H:
            ].T.astype(ml_dtypes.bfloat16)
        aux = np.zeros((H, 2), dtype=np.float32)
        aux[: 128 * OC, 0] = fc_b[h0 : h0 + 384]
        in_maps.append(dict(inp16=inp16, aux=aux))
    return in_maps


def _devcol():
    """Map triu pair index p -> device (diagonal-major padded) column."""
    colstart = np.empty(S, dtype=np.int64)
    for bi, (d0, G, L) in enumerate(BLOCKS):
        for g in range(G):
            colstart[d0 + g] = BLK_BASE[bi] + g * L
    ii, jj = np.triu_indices(S)
    return colstart[jj - ii] + ii


_DEVCOL = _devcol()


def kernel(hidden_state, fc_w, fc_b, _trace=False, **_trace_kwargs):
    from concourse.bass_utils import run_bass_kernel_spmd

    hidden_state = np.asarray(hidden_state, dtype=np.float32)
    fc_w = np.asarray(fc_w, dtype=np.float32)
    fc_b = np.asarray(fc_b, dtype=np.float32)

    in_maps = _make_in_maps(hidden_state, fc_w, fc_b)
    nc = _get_nc()
    res = run_bass_kernel_spmd(
        nc, in_maps, core_ids=list(range(8)), trace=_trace, **_trace_kwargs
    )
    LAST["res"] = res

    full = np.empty((B, H, P), dtype=np.float32)
    for k in range(8):
        b, h0 = k // 2, 384 * (k % 2)
        dev = res.results[k]["out"].reshape(384, PPAD)
        full[b, h0 : h0 + 384] = dev[:, _DEVCOL].astype(np.float32)
    return np.ascontiguousarray(full.transpose(0, 2, 1))


# revision 13
# speedup vs baseline: 1.4589x; 1.0045x over previous
"""Trainium2 Bass kernel for ConcatHandshaking.

out[b, p, :] = tanh(hidden[b, i_p] @ W1.T + hidden[b, j_p] @ W2.T + fc_b)
for the S*(S+1)/2 upper-triangular pairs (i_p, j_p), i-major order.

Device layout: output features (H=768) on SBUF partitions, pair index on the
free dim.  The pair dimension is emitted DIAGONAL-major: for diagonal
d = j - i, out(:, i, i+d) = p1T[:, i] + q2T[:, i+d] is an elementwise add of
two contiguous windows -- no broadcast operand.  G consecutive diagonals are
blocked into ONE DVE tensor_tensor via a 3D access pattern (row g reads
p1[0:L] and q2[d0+g : d0+g+L]); rows keep the max length L = 256-d0, so row
g writes g pad columns at its tail.  Everything (p1, q2, adds, tanh, output
DMA) runs in bf16: the adds qualify for the DVE 2x packed mode and the
output DMA halves vs f32.  The host drops pad columns and restores triu
order with one precomputed gather, then converts to f32.

Lead-in optimizations: the input tensor is column-grouped [ht | w_c0 | w_c1
| w_c2] so three small DMAs deliver exactly what stripe 0's matmuls need
first; stripe c+1's matmuls carry a nosync dep on stripe c's PSUM stops so
the scheduler cannot interleave all stripes k-tile-major (which would delay
stripe 0's PSUM -> first tanh by ~5us); the first two blocks are G=4 and
chunked alone so the first tanh + output DMA launch early.

Sharding (8 cores): core k handles batch b = k//2 and output-feature rows
[384*(k%2), 384*(k%2)+384) -> 3 stripes of [128 features, PPAD cols] each.
"""

import sys

import numpy as np

for _p in ("/opt/trn_rl_repo",):
    if _p not in sys.path:
        sys.path.insert(0, _p)

B, S, H = 4, 256, 768
P = S * (S + 1) // 2  # 32896
KT = H // 128  # 6 k-tiles
OC = 3  # o-chunks (of 128) per core
# bf16 packed matmul input columns: [ ht (S) | w1_c0 w2_c0 | w1_c1 w2_c1 | ... ]
IC16 = S + 2 * 128 * OC  # 1024

GPAD = 8  # q2 pad columns (max G)
# diagonal blocks (d0, G, L): two G=4 leaders for a fast first tanh, then G=8
BLOCKS = [(0, 4, 256), (4, 4, 252)] + [
    (8 + 8 * t, 8, 248 - 8 * t) for t in range(31)
]
_bases = np.concatenate([[0], np.cumsum([g * l for (_, g, l) in BLOCKS])])
BLK_BASE = _bases.astype(np.int64)
PPAD = int(BLK_BASE[-1])  # 33776

TARGET = 3500  # chunk col target for ACT + output DMA granularity


def _chunks():
    """(block_lo, block_hi, col_off, n_cols) groups; first two chunks are the
    single G=4 blocks (early first DMA), last chunk split small (short drain)."""
    chunks = [(0, 1, 0, int(BLK_BASE[1] - BLK_BASE[0])),
              (1, 2, int(BLK_BASE[1]), int(BLK_BASE[2] - BLK_BASE[1]))]
    b = 2
    while b < len(BLOCKS):
        e = b + 1
        while e < len(BLOCKS) and BLK_BASE[e] - BLK_BASE[b] < TARGET:
            e += 1
        chunks.append((b, e, int(BLK_BASE[b]), int(BLK_BASE[e] - BLK_BASE[b])))
        b = e
    # split the trailing chunk while it is large, so the final output DMA
    # (the pipeline drain) moves less than ~2000 cols
    while chunks[-1][3] > 2000:
        blo, bhi, coff, csz = chunks.pop()
        mid = blo + 1
        while BLK_BASE[mid] - BLK_BASE[blo] < csz // 2:
            mid += 1
        chunks.append((blo, mid, coff, int(BLK_BASE[mid] - BLK_BASE[blo])))
        chunks.append((mid, bhi, int(BLK_BASE[mid]), int(BLK_BASE[bhi] - BLK_BASE[mid])))
    return chunks


CHUNKS = _chunks()
CMAX = max(c[3] for c in CHUNKS)

_NC_CACHE = {}
LAST = {}


def _build_nc():
    import bass_rust
    import concourse.bacc as bacc
    import concourse.bass as bass
    import concourse.mybir as mybir
    import concourse.tile as tile

    def _sub_ap(t, off, dims):
        return bass.AP(tensor=t.tensor, offset=t.offset + off, ap=[t.ap[0]] + dims)

    f32 = mybir.dt.float32
    bf16 = mybir.dt.bfloat16
    nc = bacc.Bacc()

    inp16_d = nc.declare_dram_parameter("inp16", [H, IC16], bf16, isOutput=False)
    # f32 side data: col 0 = fcb (rows 0:384), col 1 = zeros
    aux_d = nc.declare_dram_parameter("aux", [H, 2], f32, isOutput=False)
    out_d = nc.declare_dram_parameter("out", [OC, 128, PPAD], bf16, isOutput=True)

    Tanh = mybir.ActivationFunctionType.Tanh

    with tile.TileContext(nc) as tc:
        with (
            tc.tile_pool(name="const", bufs=1) as cpool,
            tc.tile_pool(name="mm", bufs=4, space="PSUM") as mpool,
            tc.tile_pool(name="outp", bufs=6) as opool,
            tc.tile_pool(name="outp2", bufs=6) as opool2,
        ):
            inp_b = cpool.tile([128, KT * IC16], bf16, name="inp_b")
            inp_r = inp_b[:].rearrange("p (t c) -> p t c", t=KT)
            src_r = inp16_d.rearrange("(t p) c -> p t c", p=128)
            # part A: ht + stripe-0 weights, two k-tiles per DMA so PE can
            # start as soon as the first pair lands
            for kk in range(0, KT, 2):
                nc.sync.dma_start(
                    inp_r[:, kk : kk + 2, 0:512], src_r[:, kk : kk + 2, 0:512]
                )
            aux_b = cpool.tile([128, KT * 2], f32, name="aux_b")
            nc.sync.dma_start(
                aux_b[:].rearrange("p (t c) -> p t c", t=KT),
                aux_d.rearrange("(t p) c -> p t c", p=128),
            )
            # part B: stripe 1-2 weights, one DMA
            nc.sync.dma_start(
                inp_r[:, :, 512:IC16], src_r[:, :, 512:IC16]
            )

            ht_t = [inp_b[:, kk * IC16 : kk * IC16 + S] for kk in range(KT)]
            fcb_t = [aux_b[:, c * 2 : c * 2 + 1] for c in range(OC)]

            prev_stops = []
            for c in range(OC):
                w1c = S + 256 * c
                w2c = S + 256 * c + 128
                pm1 = mpool.tile([128, S], f32, name="pm1")
                # pm2 carries 4 pad columns: stripe 0's first two chunks are
                # computed straight from PSUM (scalar_tensor_tensor) and the
                # blocked window reads run up to col S+3
                pm2 = mpool.tile([128, S + 4], f32, name="pm2")
                if c == 0:
                    nc.vector.memset(pm2[:, S : S + 4], 0.0)
                stops = []
                for pm, wc in ((pm1, w1c), (pm2, w2c)):
                    for kk in range(KT):
                        mm = nc.tensor.matmul(
                            pm[:, :S],
                            inp_b[:, kk * IC16 + wc : kk * IC16 + wc + 128],
                            ht_t[kk],
                            start=(kk == 0),
                            stop=(kk == KT - 1),
                        )
                        if kk == 0 and prev_stops:
                            # keep PE stripe-major: without this the scheduler
                            # interleaves all stripes k-tile-major and stripe
                            # 0's PSUM stop retires ~5us late
                            deps = bass_rust.InstructionNameOrderedSet()
                            for nm in prev_stops:
                                deps.add(nm)
                            mm.ins.add_nosync_dependencies_from(deps)
                        if kk == KT - 1:
                            stops.append(mm.ins.name)
                prev_stops = stops

                p1 = cpool.tile([128, S], bf16, name=f"p1_{c}")
                q2 = cpool.tile([128, S + GPAD], bf16, name=f"q2_{c}")
                nc.vector.memset(q2[:, S : S + GPAD], 0.0)
                nc.vector.tensor_copy(p1[:], pm1[:, :S])

                if c != 0:
                    nc.vector.tensor_scalar_add(q2[:, :S], pm2[:, :S], fcb_t[c])
                for ci, (blo, bhi, coff, csz) in enumerate(CHUNKS):
                    ot = opool.tile([128, CMAX], bf16, name="ot")
                    for bb in range(blo, bhi):
                        d0, G, L = BLOCKS[bb]
                        off = int(BLK_BASE[bb]) - coff
                        if c == 0 and bb < 1:
                            # chunk 0 straight off PSUM pm2 (+p1, +fcb): the
                            # q2 bias pass leaves the first-tanh critical path
                            nc.vector.scalar_tensor_tensor(
                                _sub_ap(ot, off, [[L, G], [1, L]]),
                                _sub_ap(pm2, d0, [[1, G], [1, L]]),
                                fcb_t[c],
                                _sub_ap(p1, 0, [[0, G], [1, L]]),
                                mybir.AluOpType.add,
                                mybir.AluOpType.add,
                            )
                        else:
                            nc.vector.tensor_tensor(
                                _sub_ap(ot, off, [[L, G], [1, L]]),
                                _sub_ap(p1, 0, [[0, G], [1, L]]),
                                _sub_ap(q2, d0, [[1, G], [1, L]]),
                                op=mybir.AluOpType.add,
                            )
                    if c == 0 and ci == 0:
                        nc.vector.tensor_scalar_add(
                            q2[:, :S], pm2[:, :S], fcb_t[c]
                        )
                    ot2 = opool2.tile([128, CMAX], bf16, name="ot2")
                    nc.scalar.activation(ot2[:, :csz], ot[:, :csz], Tanh)
                    # the very last DMA issues from the ACT sequencer (idle
                    # right after its final tanh) instead of queueing on SP
                    dma_eng = (
                        nc.scalar
                        if (c == OC - 1 and ci == len(CHUNKS) - 1)
                        else nc.sync
                    )
                    dma_eng.dma_start(out_d[c, :, coff : coff + csz], ot2[:, :csz])
    nc.compile()
    return nc


def _get_nc():
    if "nc" not in _NC_CACHE:
        _NC_CACHE["nc"] = _build_nc()
    return _NC_CACHE["nc"]


def _make_in_maps(hidden_state, fc_w, fc_b):
    import ml_dtypes

    in_maps = []
    for k in range(8):
        b, h0 = k // 2, 384 * (k % 2)
        inp16 = np.empty((H, IC16), dtype=ml_dtypes.bfloat16)
        inp16[:, :S] = hidden_state[b].T.astype(ml_dtypes.bfloat16)
        for c in range(OC):
            r0 = h0 + 128 * c
            inp16[:, S + 256 * c : S + 256 * c + 128] = fc_w[
                r0 : r0 + 128, :H
            ].T.astype(ml_dtypes.bfloat16)
            inp16[:, S + 256 * c + 128 : S + 256 * c + 256] = fc_w[
                r0 : r0 + 128, H:
            ].T.astype(ml_dtypes.bfloat16)
        aux = np.zeros((H, 2), dtype=np.float32)
        aux[: 128 * OC, 0] = fc_b[h0 : h0 + 384]
        in_maps.append(dict(inp16=inp16, aux=aux))
    return in_maps


def _devcol():
    """Map triu pair index p -> device (diagonal-major padded) column."""
    colstart = np.empty(S, dtype=np.int64)
    for bi, (d0, G, L) in enumerate(BLOCKS):
        for g in range(G):
            colstart[d0 + g] = BLK_BASE[bi] + g * L
    ii, jj = np.triu_indices(S)
    return colstart[jj - ii] + ii


_DEVCOL = _devcol()


def kernel(hidden_state, fc_w, fc_b, _trace=False, **_trace_kwargs):
    from concourse.bass_utils import run_bass_kernel_spmd

    hidden_state = np.asarray(hidden_state, dtype=np.float32)
    fc_w = np.asarray(fc_w, dtype=np.float32)
    fc_b = np.asarray(fc_b, dtype=np.float32)

    in_maps = _make_in_maps(hidden_state, fc_w, fc_b)
    nc = _get_nc()
    res = run_bass_kernel_spmd(
        nc, in_maps, core_ids=list(range(8)), trace=_trace, **_trace_kwargs
    )
    LAST["res"] = res

    full = np.empty((B, H, P), dtype=np.float32)
    for k in range(8):
        b, h0 = k // 2, 384 * (k % 2)
        dev = res.results[k]["out"].reshape(384, PPAD)
        full[b, h0 : h0 + 384] = dev[:, _DEVCOL].astype(np.float32)
    return np.ascontiguousarray(full.transpose(0, 2, 1))
